# revision 22
# baseline (speedup 1.0000x reference)
"""Trainium2 Bass kernel v5 for nn_EnhancedLossModule.

Per-core plan (8 cores, 256 rows each, SPMD-uniform program):
  - Host precomputes fp8(e4m3) features Q, normalized Qn, exact row
    norms r of Q, and all same-label pair dot products (thresholds for
    the triplet pair reductions + contrastive pair corrections).
  - d2 = r_i + r_j - 2*Q_i.Q_j via fp8 DoubleRow matmuls (0.5 cyc/row)
    + one fp16 rank-1 matmul (ones x r_row) + a tiny one-hot matmul
    that adds 4096 to each row's own column (diag mask, NaN-safe sqrt).
    Columns are rotated by c*R per core so the diag block is always in
    the first 256 columns -> the program is identical on every core.
  - PSUM is used as [128, 1024] half-tiles (2 banks each, 4 in flight)
    so PE streams without bank stalls; warm-up matmuls ramp the PE
    p-state before the real work arrives.
  - ACT does sqrt(psum + r_i) -> dpt fp16 and the focal exp pass ->
    exactly 2 activation-table loads.
  - Triplet reductions: sum_n min(d', x) via DVE tensor_scalar
    min+accum passes: threshold 1.0 (self term) plus S0/S1 per-anchor
    threshold columns (rows sorted by partner count so the heavy
    anchors share tile m0).
  - sim = Qn_i.Qn_j via fp8 DoubleRow matmuls; sum_n min(sim, 0.5)
    reduced straight from PSUM halves (DVE).
  - Focal/label-smoothing: HW computes per-row sum(exp(pred_bf16));
    host does the O(B) log/pow tail.
"""

import os

import ml_dtypes
import numpy as np

import concourse.bacc as bacc
import concourse.bass as bass
import concourse.tile as tile
from concourse import mybir
from concourse.bass_utils import run_bass_kernel_spmd

B, C, D = 2048, 1000, 512
N_CORES = 8
R = B // N_CORES            # 256 rows per core
RT = R // 128               # 2 row tiles
KT = D // 128               # 4 contraction tiles (2 DoubleRow pairs)
HB = B // 2                 # psum half-tile width

TEMPERATURE = 0.07
C_MARGIN = 0.5
T_MARGIN = 1.0
GAMMA = 2.0
ALPHA = 0.25
SMOOTHING = 0.1
W_CONTRASTIVE = 0.1
W_TRIPLET = 0.1
W_FOCAL = 0.4
W_LABEL_SMOOTH = 0.4

DIAG = 4096.0               # added to d2 of each row's own column
OFF = SMOOTHING / (C - 1)
WARMN = int(os.environ.get("WARMN", "19"))
WARM2 = int(os.environ.get("WARM2", "2"))
EXPWAIT = float(os.environ.get("EXPWAIT_MS", "0.0095"))

F32 = mybir.dt.float32
BF16 = mybir.dt.bfloat16
FP16 = mybir.dt.float16
F8E4 = mybir.dt.float8e4
ALU = mybir.AluOpType
AF = mybir.ActivationFunctionType
E4M3 = ml_dtypes.float8_e4m3fn

_BUILD_CACHE: dict = {}


def _ap3(t, off, d1, n1, d2, n2):
    """3-dim AP view of a 2-D tile: [[pstride,128],[d1,n1],[d2,n2]]."""
    a = t[:, :]
    return bass.AP(tensor=a.tensor, offset=a.offset + off,
                   ap=[[a.ap[0][0], 128], [d1, n1], [d2, n2]])


def _build(S0: int, S1: int):
    key = (S0, S1)
    if key in _BUILD_CACHE:
        return _BUILD_CACHE[key]
    SD = S0 + S1

    # accumulator columns
    C_SELF = 0                  # RT: sum min(d', 1) per row tile
    C_DIR = C_SELF + RT         # SD: per-anchor pair sums
    C_SIM = C_DIR + SD          # 2*RT: sum min(sim, 0.5) per half
    C_SE = C_SIM + 2 * RT       # RT: sum exp(pred)
    C_SELF2 = C_SE + RT         # 2: h1 halves of split self/s0 passes
    NCOL = C_SELF2 + 2

    nc = bacc.Bacc("TRN2", target_bir_lowering=False, debug=False,
                   num_devices=N_CORES)

    ft8 = nc.dram_tensor("ft8", [128, KT * B], F8E4, kind="ExternalInput")
    fn8 = nc.dram_tensor("fn8", [128, KT * B], F8E4, kind="ExternalInput")
    fl8 = nc.dram_tensor("fl8", [128, KT * R], F8E4, kind="ExternalInput")
    fnl8 = nc.dram_tensor("fnl8", [128, KT * R], F8E4,
                          kind="ExternalInput")
    rrow = nc.dram_tensor("rrow", [1, B], FP16, kind="ExternalInput")
    pred2 = nc.dram_tensor("pred2", [128, RT * C], BF16,
                           kind="ExternalInput")
    # aux f32: [rloc RT][colx RT][xdir SD]
    NAUX = 2 * RT + SD
    aux = nc.dram_tensor("aux", [128, NAUX], F32, kind="ExternalInput")
    acc_out = nc.dram_tensor("acc_out", [128, NCOL], F32,
                             kind="ExternalOutput")
    A_RLOC, A_COLX, A_XDIR = 0, RT, 2 * RT

    with tile.TileContext(nc) as tc:
        with (
            tc.tile_pool(name="persist", bufs=1) as persist,
            tc.tile_pool(name="gpsum", bufs=4, space="PSUM") as gpsum,
        ):
            # ---------------- inputs ----------------
            iota256 = persist.tile([128, 256], F32)
            nc.gpsimd.iota(iota256, pattern=[[1, 256]], base=0,
                           channel_multiplier=0,
                           allow_small_or_imprecise_dtypes=True)
            pid = persist.tile([128, 1], F32)
            nc.gpsimd.iota(pid, pattern=[[0, 1]], base=0,
                           channel_multiplier=1,
                           allow_small_or_imprecise_dtypes=True)
            rro = persist.tile([1, B], FP16)
            nc.scalar.dma_start(out=rro, in_=rrow.ap())
            fln = persist.tile([128, 2 * KT * R], F8E4)
            nc.gpsimd.dma_start(out=fln[:, :KT * R], in_=fl8.ap())
            auxt = persist.tile([128, NAUX], F32)
            nc.gpsimd.dma_start(out=auxt, in_=aux.ap())
            nc.gpsimd.dma_start(out=fln[:, KT * R:], in_=fnl8.ap())
            fl = fln[:, :KT * R]
            fnl = fln[:, KT * R:]

            ft = [persist.tile([128, 2 * B], F8E4, name=f"ftk{kp}")
                  for kp in range(KT // 2)]
            fn = [persist.tile([128, 2 * B], F8E4, name=f"fnk{kp}")
                  for kp in range(KT // 2)]

            def qdma(dst, dsrc, kp, h):
                s = dsrc.ap()
                src_ap = bass.AP(
                    tensor=s.tensor, offset=s.offset + 2 * kp * B + h * HB,
                    ap=[[s.ap[0][0], 128], [B, 2], [1, HB]])
                d = dst[kp][:, :]
                dst_ap = bass.AP(
                    tensor=d.tensor, offset=d.offset + h * HB,
                    ap=[[d.ap[0][0], 128], [B, 2], [1, HB]])
                nc.sync.dma_start(out=dst_ap, in_=src_ap)

            qdma(ft, ft8, 0, 0)
            qdma(ft, ft8, 1, 0)
            qdma(ft, ft8, 0, 1)
            qdma(ft, ft8, 1, 1)
            qdma(fn, fn8, 0, 0)
            qdma(fn, fn8, 1, 0)
            qdma(fn, fn8, 0, 1)
            qdma(fn, fn8, 1, 1)
            pr2 = persist.tile([128, RT * C], BF16)
            nc.sync.dma_start(out=pr2, in_=pred2.ap())

            # ---------------- constants ----------------
            ident = persist.tile([128, 128], FP16)
            nc.vector.tensor_scalar(out=ident, in0=iota256[:, 0:128],
                                    scalar1=pid, scalar2=None,
                                    op0=ALU.is_equal)
            ones1 = persist.tile([1, 128], FP16)
            nc.vector.memset(ones1, 1.0)
            pm = persist.tile([128, RT * 256], FP16)
            for m in range(RT):
                nc.vector.tensor_scalar(
                    out=pm[:, m * 256:(m + 1) * 256], in0=iota256,
                    scalar1=auxt[:, A_COLX + m:A_COLX + m + 1],
                    scalar2=DIAG, op0=ALU.is_equal, op1=ALU.mult)
            halfc = persist.tile([128, 1], F32)
            nc.vector.memset(halfc, C_MARGIN)
            junk_v = persist.tile([128, B], FP16)
            junk_v2 = persist.tile([128, B], FP16)
            junk_w = persist.tile([128, HB], FP16)
            junk_a = persist.tile([128, C], BF16)
            acc = persist.tile([128, NCOL], F32)
            nc.vector.memset(acc, 0.0)
            dpt = [persist.tile([128, B], FP16, name=f"dpt{m}")
                   for m in range(RT)]
            # early dummy sqrt binds the sqrt-table load to idle time
            tiny = persist.tile([128, 1], FP16)
            nc.scalar.activation(out=tiny, in_=pid, func=AF.Sqrt)

            # ---------------- matmuls ----------------
            def mm_kp(ps, srck, m, h, kp, lo, start, stop):
                for ch in range(2):
                    o = ch * 512
                    nc.tensor.matmul(
                        ps[:, o:o + 512],
                        _ap3(fln, lo + 2 * kp * R + m * 128, R, 2, 1, 128),
                        _ap3(srck[kp], h * HB + o, B, 2, 1, 512),
                        start=start, stop=stop,
                        perf_mode=mybir.MatmulPerfMode.DoubleRow,
                        skip_group_check=True,
                    )

            def mm_finish(ps, m, h):
                if h == 0:
                    nc.tensor.matmul(
                        ps[:, 0:256], ident[:, :],
                        pm[:, m * 256:(m + 1) * 256],
                        start=False, stop=False, skip_group_check=True,
                    )
                for ch in range(2):
                    o = ch * 512
                    nc.tensor.matmul(
                        ps[:, o:o + 512], ones1[0:1, :],
                        rro[0:1, h * HB + o:h * HB + o + 512],
                        start=False, stop=True, skip_group_check=True,
                    )

            FLO = 0
            FNO = KT * R
            d2ps = [[gpsum.tile([128, HB], F32, tag="big",
                                name=f"d2ps{m}{h}") for h in range(2)]
                    for m in range(RT)]
            simps = [[None] * 2 for _ in range(RT)]

            for w in range(WARMN):
                nc.tensor.matmul(d2ps[0][1][:, 0:128], ident[:, :],
                                 ident[:, :], start=True, stop=True,
                                 skip_group_check=True)
            for m in range(RT):
                mm_kp(d2ps[m][0], ft, m, 0, 0, FLO, True, False)
            for w in range(WARM2):
                nc.tensor.matmul(d2ps[0][1][:, 0:128], ident[:, :],
                                 ident[:, :], start=True, stop=True,
                                 skip_group_check=True)
            for m in range(RT):
                mm_kp(d2ps[m][0], ft, m, 0, 1, FLO, False, False)
                mm_finish(d2ps[m][0], m, 0)
            for m in range(RT):
                mm_kp(d2ps[m][1], ft, m, 1, 0, FLO, True, False)
            for m in range(RT):
                mm_kp(d2ps[m][1], ft, m, 1, 1, FLO, False, False)
                mm_finish(d2ps[m][1], m, 1)
            for h in range(2):
                for m in range(RT):
                    simps[m][h] = gpsum.tile([128, HB], F32, tag="big",
                                             name=f"simps{m}{h}")
            for m in range(RT):
                mm_kp(simps[m][0], fn, m, 0, 0, FNO, True, False)
            for m in range(RT):
                mm_kp(simps[m][0], fn, m, 0, 1, FNO, False, True)
            for m in range(RT):
                mm_kp(simps[m][1], fn, m, 1, 0, FNO, True, False)
            for m in range(RT):
                mm_kp(simps[m][1], fn, m, 1, 1, FNO, False, True)

            # ---------------- ACT: sqrts, exps, relus ------------------
            for m in range(RT):
                for h in range(2):
                    nc.scalar.activation(
                        out=dpt[m][:, h * HB:(h + 1) * HB],
                        in_=d2ps[m][h], func=AF.Sqrt,
                        bias=auxt[:, A_RLOC + m:A_RLOC + m + 1])
            for m in range(RT):
                nc.scalar.activation(
                    out=junk_a, in_=pr2[:, m * C:(m + 1) * C],
                    func=AF.Exp,
                    accum_out=acc[:, C_SE + m:C_SE + m + 1])
            for m in range(RT):
                nc.scalar.activation(
                    out=junk_w, in_=simps[m][1], func=AF.Relu,
                    scale=-1.0, bias=halfc[:, 0:1],
                    accum_out=acc[:, C_SIM + 2 + m:C_SIM + 2 + m + 1])

            # ---------------- DVE: triplet + sim reductions ------------
            jv = [junk_v, junk_v2]
            ji = 0
            # first two m0 passes split by half so DVE starts on dpt0-h0
            nc.vector.tensor_scalar(
                out=junk_v[:, 0:HB], in0=dpt[0][:, 0:HB],
                scalar1=float(T_MARGIN), scalar2=None,
                op0=ALU.min, op1=ALU.add,
                accum_out=acc[:, C_SELF:C_SELF + 1])
            nc.vector.tensor_scalar(
                out=junk_v2[:, 0:HB], in0=dpt[0][:, 0:HB],
                scalar1=auxt[:, A_XDIR:A_XDIR + 1],
                scalar2=None, op0=ALU.min, op1=ALU.add,
                accum_out=acc[:, C_DIR:C_DIR + 1])
            nc.vector.tensor_scalar(
                out=junk_v[:, 0:HB], in0=dpt[0][:, HB:B],
                scalar1=float(T_MARGIN), scalar2=None,
                op0=ALU.min, op1=ALU.add,
                accum_out=acc[:, C_SELF2:C_SELF2 + 1])
            nc.vector.tensor_scalar(
                out=junk_v2[:, 0:HB], in0=dpt[0][:, HB:B],
                scalar1=auxt[:, A_XDIR:A_XDIR + 1],
                scalar2=None, op0=ALU.min, op1=ALU.add,
                accum_out=acc[:, C_SELF2 + 1:C_SELF2 + 2])
            for m in range(RT):
                if m > 0:
                    nc.vector.tensor_scalar(
                        out=jv[ji % 2], in0=dpt[m],
                        scalar1=float(T_MARGIN),
                        scalar2=None, op0=ALU.min, op1=ALU.add,
                        accum_out=acc[:, C_SELF + m:C_SELF + m + 1])
                    ji += 1
                ns = S0 if m == 0 else S1
                off = 0 if m == 0 else S0
                for s in range(ns):
                    if m == 0 and s == 0:
                        continue
                    nc.vector.tensor_scalar(
                        out=jv[ji % 2], in0=dpt[m],
                        scalar1=auxt[:, A_XDIR + off + s:
                                     A_XDIR + off + s + 1],
                        scalar2=None, op0=ALU.min, op1=ALU.add,
                        accum_out=acc[:, C_DIR + off + s:
                                      C_DIR + off + s + 1])
                    ji += 1
            for m in range(RT):
                nc.vector.tensor_scalar(
                    out=jv[ji % 2][:, 0:HB], in0=simps[m][0],
                    scalar1=float(C_MARGIN), scalar2=None,
                    op0=ALU.min, op1=ALU.add,
                    accum_out=acc[:, C_SIM + m:C_SIM + m + 1])
                ji += 1

            nc.sync.dma_start(out=acc_out.ap(), in_=acc)

    nc.compile()
    meta = dict(S0=S0, S1=S1, NCOL=NCOL, C_SELF=C_SELF, C_DIR=C_DIR,
                C_SIM=C_SIM, C_SE=C_SE, C_SELF2=C_SELF2)
    _BUILD_CACHE[key] = (nc, meta)
    return nc, meta


def _host_prep(pred, target, features):
    pred = np.asarray(pred, dtype=np.float64)
    lab = np.asarray(target).astype(np.int64)
    f = np.asarray(features, dtype=np.float64)

    Q = f.astype(np.float32).astype(E4M3)
    Qf = Q.astype(np.float64)
    r = np.einsum("ij,ij->i", Qf, Qf)                  # exact fp8 row norms
    rr16 = r.astype(np.float16)
    rr16f = rr16.astype(np.float64)

    nrm = np.linalg.norm(f, axis=1)
    n = f / nrm[:, None]
    Qn = n.astype(np.float32).astype(E4M3)
    Qnf = Qn.astype(np.float64)

    # same-label partner lists
    order = np.argsort(lab, kind="stable")
    sl = lab[order]
    starts = np.flatnonzero(np.r_[True, sl[1:] != sl[:-1]])
    ends = np.r_[starts[1:], len(sl)]
    partners = [[] for _ in range(B)]
    for s, e in zip(starts, ends):
        if e - s < 2:
            continue
        mem = order[s:e]
        for a in mem:
            for p in mem:
                if p != a:
                    partners[a].append(int(p))
    pcnt = np.array([len(p) for p in partners])
    NP = int(pcnt.sum())

    # in-core sort by partner count (heavy anchors into tile m0)
    corder = np.empty((N_CORES, R), np.int64)
    for c in range(N_CORES):
        seg = np.arange(c * R, (c + 1) * R)
        corder[c] = seg[np.argsort(-pcnt[seg], kind="stable")]

    S0 = max(1, int(max(pcnt[corder[c][:128]].max()
                        for c in range(N_CORES))))
    S1 = max(1, int(max(pcnt[corder[c][128:]].max()
                        for c in range(N_CORES))))

    # pair values (Q-space distances; original/Qn-space sims)
    dq = [None] * B
    sim_true = [None] * B
    sim_q = [None] * B
    for a in range(B):
        ps = partners[a]
        if not ps:
            continue
        P = np.array(ps)
        g = Qf[P] @ Qf[a]
        d2 = r[a] + rr16f[P] - 2.0 * g
        dq[a] = np.sqrt(np.maximum(d2, 0.0))
        sim_true[a] = (f[P] @ f[a]) / (nrm[a] * nrm[P])
        sim_q[a] = Qnf[P] @ Qnf[a]

    SD = S0 + S1
    in_maps = []
    xdir_all = np.zeros((N_CORES, 128, SD), np.float64)
    dir_map = [[] for _ in range(N_CORES)]  # (p, m, s, anchor)

    ftT = np.ascontiguousarray(Q.T)          # [D, B]
    fnT = np.ascontiguousarray(Qn.T)

    for c in range(N_CORES):
        rot = np.roll(np.arange(B), -c * R)  # rotated column order
        ft8 = np.empty((128, KT * B), E4M3)
        fn8 = np.empty((128, KT * B), E4M3)
        for k in range(KT):
            ft8[:, k * B:(k + 1) * B] = ftT[k * 128:(k + 1) * 128, rot]
            fn8[:, k * B:(k + 1) * B] = fnT[k * 128:(k + 1) * 128, rot]
        rows = corder[c]
        fl8 = np.empty((128, KT * R), E4M3)
        fnl8 = np.empty((128, KT * R), E4M3)
        m2q = (-2.0 * Qf[rows]).astype(E4M3)      # exact in e4m3
        for k in range(KT):
            fl8[:, k * R:(k + 1) * R] = m2q[:, k * 128:(k + 1) * 128].T
            fnl8[:, k * R:(k + 1) * R] = \
                fnT[k * 128:(k + 1) * 128][:, rows]
        rrow_h = rr16[rot][None, :]

        auxh = np.zeros((128, 2 * RT + SD), np.float32)
        for m in range(RT):
            Sm = S0 if m == 0 else S1
            soff = 0 if m == 0 else S0
            for p in range(128):
                g = int(rows[m * 128 + p])
                auxh[p, m] = r[g]                     # rloc (fp32 bias)
                auxh[p, RT + m] = (g - c * R) % B     # colx (rotated)
                ps = partners[g]
                for s in range(min(len(ps), Sm)):
                    x = dq[g][s] + T_MARGIN
                    auxh[p, 2 * RT + soff + s] = x
                    xdir_all[c, p, soff + s] = x
                    dir_map[c].append((p, m, s, g))

        pr = np.asarray(pred, np.float32)[c * R:(c + 1) * R]
        pr = pr.astype(ml_dtypes.bfloat16).reshape(RT, 128, C)
        pr2 = np.ascontiguousarray(pr.transpose(1, 0, 2).reshape(
            128, RT * C))

        in_maps.append({
            "ft8": ft8, "fn8": fn8, "fl8": fl8, "fnl8": fnl8,
            "rrow": rrow_h, "pred2": pr2, "aux": auxh,
        })

    prep = dict(S0=S0, S1=S1, NP=NP, partners=partners, pcnt=pcnt,
                corder=corder, dq=dq, sim_true=sim_true, sim_q=sim_q,
                dir_map=dir_map, xdir=xdir_all, pred=pred, lab=lab)
    return in_maps, prep


def _combine(results, meta, prep):
    accs = np.stack([r["acc_out"] for r in results]).astype(np.float64)
    S0 = prep["S0"]
    dq, sim_true, sim_q = prep["dq"], prep["sim_true"], prep["sim_q"]
    NP = prep["NP"]
    Bf = float(B)

    # ---- contrastive ----
    pos_pair = sum(
        -np.log(np.exp(np.asarray(sim_true[a]) / TEMPERATURE) + 1e-8).sum()
        for a in range(B) if sim_true[a] is not None)
    pos_sum = (pos_pair
               + B * (-np.log(np.exp(1.0 / TEMPERATURE) + 1e-8))
               + (Bf * Bf - B - NP) * (-np.log1p(1e-8)))

    M = (accs[:, :, meta["C_SIM"]:meta["C_SIM"] + 2].sum()
         + (C_MARGIN * HB * 2 * 128 * N_CORES
            - accs[:, :, meta["C_SIM"] + 2:meta["C_SIM"] + 4].sum()))
    pair_min = sum(np.minimum(np.asarray(sim_q[a]), C_MARGIN).sum()
                   for a in range(B) if sim_q[a] is not None)
    sum_min_diff = M - C_MARGIN * B - pair_min
    neg_sum = 0.5 * Bf * Bf - sum_min_diff
    lc = (pos_sum + neg_sum) / (Bf * Bf)

    # ---- triplet self term ----
    selfsum = (accs[:, :, meta["C_SELF"]:meta["C_SELF"] + RT].sum()
               + accs[:, :, meta["C_SELF2"]].sum())
    mp = sum(np.minimum(dq[a], T_MARGIN).sum()
             for a in range(B) if dq[a] is not None)
    n_diff_sum = Bf * Bf - (B + NP)
    self_part = n_diff_sum * T_MARGIN - (selfsum - B * T_MARGIN - mp)

    # ---- triplet pair term ----
    pair_part = 0.0
    for c in range(N_CORES):
        acc_c = accs[c]
        for (p, m, s, a) in prep["dir_map"][c]:
            x = prep["xdir"][c, p, (0 if m == 0 else S0) + s]
            S_ap = acc_c[p, meta["C_DIR"] + (0 if m == 0 else S0) + s]
            if m == 0 and s == 0:
                S_ap += acc_c[p, meta["C_SELF2"] + 1]
            corr = np.maximum(x - dq[a], 0.0).sum()
            pair_part += Bf * x - S_ap - corr
    lt = (self_part + pair_part) / (Bf + 1e-8)

    # ---- focal + label smoothing (host tail) ----
    pred, lab = prep["pred"], prep["lab"]
    se = np.empty(B)
    for c in range(N_CORES):
        for m in range(RT):
            se[c * R + m * 128:c * R + (m + 1) * 128] = \
                accs[c, :, meta["C_SE"] + m]
    lse = np.log(se)
    ptgt = pred[np.arange(B), lab]
    spred = pred.sum(axis=1)
    ce = lse - ptgt
    pt = np.exp(-ce)
    lf = (ALPHA * (1.0 - pt) ** GAMMA * ce).mean()
    ls = (-(OFF * (spred - C * lse)
            + (1.0 - SMOOTHING - OFF) * (ptgt - lse))).mean()

    total = (W_CONTRASTIVE * lc + W_TRIPLET * lt
             + W_FOCAL * lf + W_LABEL_SMOOTH * ls)
    return np.array([lc, lt, lf, ls, total], dtype=np.float32)


def kernel(pred, target, features):
    in_maps, prep = _host_prep(pred, target, features)
    nc, meta = _build(prep["S0"], prep["S1"])
    res = run_bass_kernel_spmd(nc, in_maps, core_ids=list(range(N_CORES)))
    return _combine(res.results, meta, prep)


if __name__ == "__main__":
    import reference

    inputs = reference.setup_inputs()
    expected = np.asarray(reference.reference(**inputs))
    actual = kernel(**{k: np.asarray(v) for k, v in inputs.items()})
    err = np.abs(actual - expected) / np.maximum(np.abs(expected), 1e-12)
    print("expected:", expected)
    print("actual:  ", actual)
    print("rel err: ", err)


# revision 24
# speedup vs baseline: 1.0549x; 1.0549x over previous
"""Trainium2 Bass kernel v5 for nn_EnhancedLossModule.

Per-core plan (8 cores, 256 rows each, SPMD-uniform program):
  - Host precomputes fp8(e4m3) features Q, normalized Qn, exact row
    norms r of Q, and all same-label pair dot products (thresholds for
    the triplet pair reductions + contrastive pair corrections).
  - d2 = r_i + r_j - 2*Q_i.Q_j via fp8 DoubleRow matmuls (0.5 cyc/row)
    + one fp16 rank-1 matmul (ones x r_row) + a tiny one-hot matmul
    that adds 4096 to each row's own column (diag mask, NaN-safe sqrt).
    Columns are rotated by c*R per core so the diag block is always in
    the first 256 columns -> the program is identical on every core.
  - PSUM is used as [128, 1024] half-tiles (2 banks each, 4 in flight)
    so PE streams without bank stalls; warm-up matmuls ramp the PE
    p-state before the real work arrives.
  - ACT does sqrt(psum + r_i) -> dpt fp16 and the focal exp pass ->
    exactly 2 activation-table loads.
  - Triplet reductions: sum_n min(d', x) via DVE tensor_scalar
    min+accum passes: threshold 1.0 (self term) plus S0/S1 per-anchor
    threshold columns (rows sorted by partner count so the heavy
    anchors share tile m0).
  - sim = Qn_i.Qn_j via fp8 DoubleRow matmuls; sum_n min(sim, 0.5)
    reduced straight from PSUM halves (DVE).
  - Focal/label-smoothing: HW computes per-row sum(exp(pred_bf16));
    host does the O(B) log/pow tail.
"""

import os

import ml_dtypes
import numpy as np

import concourse.bacc as bacc
import concourse.bass as bass
import concourse.tile as tile
from concourse import mybir
from concourse.bass_utils import run_bass_kernel_spmd

B, C, D = 2048, 1000, 512
N_CORES = 8
R = B // N_CORES            # 256 rows per core
RT = R // 128               # 2 row tiles
KT = D // 128               # 4 contraction tiles (2 DoubleRow pairs)
HB = B // 2                 # psum half-tile width

TEMPERATURE = 0.07
C_MARGIN = 0.5
T_MARGIN = 1.0
GAMMA = 2.0
ALPHA = 0.25
SMOOTHING = 0.1
W_CONTRASTIVE = 0.1
W_TRIPLET = 0.1
W_FOCAL = 0.4
W_LABEL_SMOOTH = 0.4

DIAG = 4096.0               # added to d2 of each row's own column
OFF = SMOOTHING / (C - 1)
WARMN = int(os.environ.get("WARMN", "22"))
WARM2 = int(os.environ.get("WARM2", "2"))
EXPWAIT = float(os.environ.get("EXPWAIT_MS", "0.0095"))

F32 = mybir.dt.float32
BF16 = mybir.dt.bfloat16
FP16 = mybir.dt.float16
F8E4 = mybir.dt.float8e4
ALU = mybir.AluOpType
AF = mybir.ActivationFunctionType
E4M3 = ml_dtypes.float8_e4m3fn

_BUILD_CACHE: dict = {}


def _ap3(t, off, d1, n1, d2, n2):
    """3-dim AP view of a 2-D tile: [[pstride,128],[d1,n1],[d2,n2]]."""
    a = t[:, :]
    return bass.AP(tensor=a.tensor, offset=a.offset + off,
                   ap=[[a.ap[0][0], 128], [d1, n1], [d2, n2]])


def _build(S0: int, S1: int):
    key = (S0, S1)
    if key in _BUILD_CACHE:
        return _BUILD_CACHE[key]
    SD = S0 + S1

    # accumulator columns
    C_SELF = 0                  # RT: sum min(d', 1) per row tile
    C_DIR = C_SELF + RT         # SD: per-anchor pair sums
    C_SIM = C_DIR + SD          # 2*RT: sum min(sim, 0.5) per half
    C_SE = C_SIM + 2 * RT       # RT: sum exp(pred)
    C_SELF2 = C_SE + RT         # 2: h1 halves of split self/s0 passes
    NCOL = C_SELF2 + 2

    nc = bacc.Bacc("TRN2", target_bir_lowering=False, debug=False,
                   num_devices=N_CORES)

    ft8 = nc.dram_tensor("ft8", [128, KT * B], F8E4, kind="ExternalInput")
    fn8 = nc.dram_tensor("fn8", [128, KT * B], F8E4, kind="ExternalInput")
    fl8 = nc.dram_tensor("fl8", [128, KT * R], F8E4, kind="ExternalInput")
    fnl8 = nc.dram_tensor("fnl8", [128, KT * R], F8E4,
                          kind="ExternalInput")
    rrow = nc.dram_tensor("rrow", [1, B], FP16, kind="ExternalInput")
    pred2 = nc.dram_tensor("pred2", [128, RT * C], BF16,
                           kind="ExternalInput")
    # aux f32: [rloc RT][colx RT][xdir SD]
    NAUX = 2 * RT + SD
    aux = nc.dram_tensor("aux", [128, NAUX], F32, kind="ExternalInput")
    acc_out = nc.dram_tensor("acc_out", [128, NCOL], F32,
                             kind="ExternalOutput")
    A_RLOC, A_COLX, A_XDIR = 0, RT, 2 * RT

    with tile.TileContext(nc) as tc:
        with (
            tc.tile_pool(name="persist", bufs=1) as persist,
            tc.tile_pool(name="gpsum", bufs=4, space="PSUM") as gpsum,
        ):
            # ---------------- inputs ----------------
            iota256 = persist.tile([128, 256], F32)
            nc.gpsimd.iota(iota256, pattern=[[1, 256]], base=0,
                           channel_multiplier=0,
                           allow_small_or_imprecise_dtypes=True)
            pid = persist.tile([128, 1], F32)
            nc.gpsimd.iota(pid, pattern=[[0, 1]], base=0,
                           channel_multiplier=1,
                           allow_small_or_imprecise_dtypes=True)
            auxt = persist.tile([128, NAUX], F32)
            nc.scalar.dma_start(out=auxt, in_=aux.ap())
            rro = persist.tile([1, B], FP16)
            nc.scalar.dma_start(out=rro, in_=rrow.ap())
            fln = persist.tile([128, 2 * KT * R], F8E4)
            nc.gpsimd.dma_start(out=fln[:, :KT * R], in_=fl8.ap())
            nc.gpsimd.dma_start(out=fln[:, KT * R:], in_=fnl8.ap())
            fl = fln[:, :KT * R]
            fnl = fln[:, KT * R:]

            ft = [persist.tile([128, 2 * B], F8E4, name=f"ftk{kp}")
                  for kp in range(KT // 2)]
            fn = [persist.tile([128, 2 * B], F8E4, name=f"fnk{kp}")
                  for kp in range(KT // 2)]

            def qdma(dst, dsrc, kp, h):
                s = dsrc.ap()
                src_ap = bass.AP(
                    tensor=s.tensor, offset=s.offset + 2 * kp * B + h * HB,
                    ap=[[s.ap[0][0], 128], [B, 2], [1, HB]])
                d = dst[kp][:, :]
                dst_ap = bass.AP(
                    tensor=d.tensor, offset=d.offset + h * HB,
                    ap=[[d.ap[0][0], 128], [B, 2], [1, HB]])
                nc.sync.dma_start(out=dst_ap, in_=src_ap)

            qdma(ft, ft8, 0, 0)
            qdma(ft, ft8, 1, 0)
            qdma(ft, ft8, 0, 1)
            qdma(ft, ft8, 1, 1)
            qdma(fn, fn8, 0, 0)
            qdma(fn, fn8, 1, 0)
            qdma(fn, fn8, 0, 1)
            qdma(fn, fn8, 1, 1)
            pr2 = persist.tile([128, RT * C], BF16)
            nc.sync.dma_start(out=pr2, in_=pred2.ap())

            # ---------------- constants ----------------
            ident = persist.tile([128, 128], FP16)
            nc.vector.tensor_scalar(out=ident, in0=iota256[:, 0:128],
                                    scalar1=pid, scalar2=None,
                                    op0=ALU.is_equal)
            ones1 = persist.tile([1, 128], FP16)
            nc.vector.memset(ones1, 1.0)
            pm = persist.tile([128, RT * 256], FP16)
            for m in range(RT):
                nc.vector.tensor_scalar(
                    out=pm[:, m * 256:(m + 1) * 256], in0=iota256,
                    scalar1=auxt[:, A_COLX + m:A_COLX + m + 1],
                    scalar2=DIAG, op0=ALU.is_equal, op1=ALU.mult)
            halfc = persist.tile([128, 1], F32)
            nc.vector.memset(halfc, C_MARGIN)
            junk_v = persist.tile([128, B], FP16)
            junk_v2 = persist.tile([128, B], FP16)
            junk_w = persist.tile([128, HB], FP16)
            junk_a = persist.tile([128, C], BF16)
            acc = persist.tile([128, NCOL], F32)
            nc.vector.memset(acc, 0.0)
            dpt = [persist.tile([128, B], FP16, name=f"dpt{m}")
                   for m in range(RT)]
            # early dummy sqrt binds the sqrt-table load to idle time
            tiny = persist.tile([128, 1], FP16)
            nc.scalar.activation(out=tiny, in_=pid, func=AF.Sqrt)

            # ---------------- matmuls ----------------
            def mm_kp(ps, srck, m, h, kp, lo, start, stop):
                for ch in range(2):
                    o = ch * 512
                    nc.tensor.matmul(
                        ps[:, o:o + 512],
                        _ap3(fln, lo + 2 * kp * R + m * 128, R, 2, 1, 128),
                        _ap3(srck[kp], h * HB + o, B, 2, 1, 512),
                        start=start, stop=stop,
                        perf_mode=mybir.MatmulPerfMode.DoubleRow,
                        skip_group_check=True,
                    )

            def mm_finish(ps, m, h):
                if h == 0:
                    nc.tensor.matmul(
                        ps[:, 0:256], ident[:, :],
                        pm[:, m * 256:(m + 1) * 256],
                        start=False, stop=False, skip_group_check=True,
                    )
                for ch in range(2):
                    o = ch * 512
                    nc.tensor.matmul(
                        ps[:, o:o + 512], ones1[0:1, :],
                        rro[0:1, h * HB + o:h * HB + o + 512],
                        start=False, stop=True, skip_group_check=True,
                    )

            FLO = 0
            FNO = KT * R
            d2ps = [[gpsum.tile([128, HB], F32, tag="big",
                                name=f"d2ps{m}{h}") for h in range(2)]
                    for m in range(RT)]
            simps = [[None] * 2 for _ in range(RT)]

            for w in range(WARMN):
                nc.tensor.matmul(d2ps[0][1][:, 0:128], ident[:, :],
                                 ident[:, :], start=True, stop=True,
                                 skip_group_check=True)
            for m in range(RT):
                mm_kp(d2ps[m][0], ft, m, 0, 0, FLO, True, False)
            for w in range(WARM2):
                nc.tensor.matmul(d2ps[0][1][:, 0:128], ident[:, :],
                                 ident[:, :], start=True, stop=True,
                                 skip_group_check=True)
            for m in range(RT):
                mm_kp(d2ps[m][0], ft, m, 0, 1, FLO, False, False)
                mm_finish(d2ps[m][0], m, 0)
            for m in range(RT):
                mm_kp(d2ps[m][1], ft, m, 1, 0, FLO, True, False)
            for m in range(RT):
                mm_kp(d2ps[m][1], ft, m, 1, 1, FLO, False, False)
                mm_finish(d2ps[m][1], m, 1)
            for h in range(2):
                for m in range(RT):
                    simps[m][h] = gpsum.tile([128, HB], F32, tag="big",
                                             name=f"simps{m}{h}")
            for m in range(RT):
                mm_kp(simps[m][0], fn, m, 0, 0, FNO, True, False)
            for m in range(RT):
                mm_kp(simps[m][0], fn, m, 0, 1, FNO, False, True)
            for m in range(RT):
                mm_kp(simps[m][1], fn, m, 1, 0, FNO, True, False)
            for m in range(RT):
                mm_kp(simps[m][1], fn, m, 1, 1, FNO, False, True)

            # ---------------- ACT: sqrts, exps, relus ------------------
            for m in range(RT):
                for h in range(2):
                    nc.scalar.activation(
                        out=dpt[m][:, h * HB:(h + 1) * HB],
                        in_=d2ps[m][h], func=AF.Sqrt,
                        bias=auxt[:, A_RLOC + m:A_RLOC + m + 1])
            for m in range(RT):
                nc.scalar.activation(
                    out=junk_a, in_=pr2[:, m * C:(m + 1) * C],
                    func=AF.Exp,
                    accum_out=acc[:, C_SE + m:C_SE + m + 1])
            for m in range(RT):
                nc.scalar.activation(
                    out=junk_w, in_=simps[m][1], func=AF.Relu,
                    scale=-1.0, bias=halfc[:, 0:1],
                    accum_out=acc[:, C_SIM + 2 + m:C_SIM + 2 + m + 1])

            # ---------------- DVE: triplet + sim reductions ------------
            jv = [junk_v, junk_v2]
            ji = 0
            # first two m0 passes split by half so DVE starts on dpt0-h0
            nc.vector.tensor_scalar(
                out=junk_v[:, 0:HB], in0=dpt[0][:, 0:HB],
                scalar1=float(T_MARGIN), scalar2=None,
                op0=ALU.min, op1=ALU.add,
                accum_out=acc[:, C_SELF:C_SELF + 1])
            nc.vector.tensor_scalar(
                out=junk_v2[:, 0:HB], in0=dpt[0][:, 0:HB],
                scalar1=auxt[:, A_XDIR:A_XDIR + 1],
                scalar2=None, op0=ALU.min, op1=ALU.add,
                accum_out=acc[:, C_DIR:C_DIR + 1])
            for m in range(RT):
                nc.vector.tensor_scalar(
                    out=jv[ji % 2][:, 0:HB], in0=simps[m][0],
                    scalar1=float(C_MARGIN), scalar2=None,
                    op0=ALU.min, op1=ALU.add,
                    accum_out=acc[:, C_SIM + m:C_SIM + m + 1])
                ji += 1
            nc.vector.tensor_scalar(
                out=junk_v[:, 0:HB], in0=dpt[0][:, HB:B],
                scalar1=float(T_MARGIN), scalar2=None,
                op0=ALU.min, op1=ALU.add,
                accum_out=acc[:, C_SELF2:C_SELF2 + 1])
            nc.vector.tensor_scalar(
                out=junk_v2[:, 0:HB], in0=dpt[0][:, HB:B],
                scalar1=auxt[:, A_XDIR:A_XDIR + 1],
                scalar2=None, op0=ALU.min, op1=ALU.add,
                accum_out=acc[:, C_SELF2 + 1:C_SELF2 + 2])
            for m in range(RT):
                if m > 0:
                    nc.vector.tensor_scalar(
                        out=jv[ji % 2], in0=dpt[m],
                        scalar1=float(T_MARGIN),
                        scalar2=None, op0=ALU.min, op1=ALU.add,
                        accum_out=acc[:, C_SELF + m:C_SELF + m + 1])
                    ji += 1
                ns = S0 if m == 0 else S1
                off = 0 if m == 0 else S0
                for s in range(ns):
                    if m == 0 and s == 0:
                        continue
                    nc.vector.tensor_scalar(
                        out=jv[ji % 2], in0=dpt[m],
                        scalar1=auxt[:, A_XDIR + off + s:
                                     A_XDIR + off + s + 1],
                        scalar2=None, op0=ALU.min, op1=ALU.add,
                        accum_out=acc[:, C_DIR + off + s:
                                      C_DIR + off + s + 1])
                    ji += 1


            nc.sync.dma_start(out=acc_out.ap(), in_=acc)

    nc.compile()
    meta = dict(S0=S0, S1=S1, NCOL=NCOL, C_SELF=C_SELF, C_DIR=C_DIR,
                C_SIM=C_SIM, C_SE=C_SE, C_SELF2=C_SELF2)
    _BUILD_CACHE[key] = (nc, meta)
    return nc, meta


def _host_prep(pred, target, features):
    pred = np.asarray(pred, dtype=np.float64)
    lab = np.asarray(target).astype(np.int64)
    f = np.asarray(features, dtype=np.float64)

    Q = f.astype(np.float32).astype(E4M3)
    Qf = Q.astype(np.float64)
    r = np.einsum("ij,ij->i", Qf, Qf)                  # exact fp8 row norms
    rr16 = r.astype(np.float16)
    rr16f = rr16.astype(np.float64)

    nrm = np.linalg.norm(f, axis=1)
    n = f / nrm[:, None]
    Qn = n.astype(np.float32).astype(E4M3)
    Qnf = Qn.astype(np.float64)

    # same-label partner lists
    order = np.argsort(lab, kind="stable")
    sl = lab[order]
    starts = np.flatnonzero(np.r_[True, sl[1:] != sl[:-1]])
    ends = np.r_[starts[1:], len(sl)]
    partners = [[] for _ in range(B)]
    for s, e in zip(starts, ends):
        if e - s < 2:
            continue
        mem = order[s:e]
        for a in mem:
            for p in mem:
                if p != a:
                    partners[a].append(int(p))
    pcnt = np.array([len(p) for p in partners])
    NP = int(pcnt.sum())

    # in-core sort by partner count (heavy anchors into tile m0)
    corder = np.empty((N_CORES, R), np.int64)
    for c in range(N_CORES):
        seg = np.arange(c * R, (c + 1) * R)
        corder[c] = seg[np.argsort(-pcnt[seg], kind="stable")]

    S0 = max(1, int(max(pcnt[corder[c][:128]].max()
                        for c in range(N_CORES))))
    S1 = max(1, int(max(pcnt[corder[c][128:]].max()
                        for c in range(N_CORES))))

    # pair values (Q-space distances; original/Qn-space sims)
    dq = [None] * B
    sim_true = [None] * B
    sim_q = [None] * B
    for a in range(B):
        ps = partners[a]
        if not ps:
            continue
        P = np.array(ps)
        g = Qf[P] @ Qf[a]
        d2 = r[a] + rr16f[P] - 2.0 * g
        dq[a] = np.sqrt(np.maximum(d2, 0.0))
        sim_true[a] = (f[P] @ f[a]) / (nrm[a] * nrm[P])
        sim_q[a] = Qnf[P] @ Qnf[a]

    SD = S0 + S1
    in_maps = []
    xdir_all = np.zeros((N_CORES, 128, SD), np.float64)
    dir_map = [[] for _ in range(N_CORES)]  # (p, m, s, anchor)

    ftT = np.ascontiguousarray(Q.T)          # [D, B]
    fnT = np.ascontiguousarray(Qn.T)

    for c in range(N_CORES):
        rot = np.roll(np.arange(B), -c * R)  # rotated column order
        ft8 = np.empty((128, KT * B), E4M3)
        fn8 = np.empty((128, KT * B), E4M3)
        for k in range(KT):
            ft8[:, k * B:(k + 1) * B] = ftT[k * 128:(k + 1) * 128, rot]
            fn8[:, k * B:(k + 1) * B] = fnT[k * 128:(k + 1) * 128, rot]
        rows = corder[c]
        fl8 = np.empty((128, KT * R), E4M3)
        fnl8 = np.empty((128, KT * R), E4M3)
        m2q = (-2.0 * Qf[rows]).astype(E4M3)      # exact in e4m3
        for k in range(KT):
            fl8[:, k * R:(k + 1) * R] = m2q[:, k * 128:(k + 1) * 128].T
            fnl8[:, k * R:(k + 1) * R] = \
                fnT[k * 128:(k + 1) * 128][:, rows]
        rrow_h = rr16[rot][None, :]

        auxh = np.zeros((128, 2 * RT + SD), np.float32)
        for m in range(RT):
            Sm = S0 if m == 0 else S1
            soff = 0 if m == 0 else S0
            for p in range(128):
                g = int(rows[m * 128 + p])
                auxh[p, m] = r[g]                     # rloc (fp32 bias)
                auxh[p, RT + m] = (g - c * R) % B     # colx (rotated)
                ps = partners[g]
                for s in range(min(len(ps), Sm)):
                    x = dq[g][s] + T_MARGIN
                    auxh[p, 2 * RT + soff + s] = x
                    xdir_all[c, p, soff + s] = x
                    dir_map[c].append((p, m, s, g))

        pr = np.asarray(pred, np.float32)[c * R:(c + 1) * R]
        pr = pr.astype(ml_dtypes.bfloat16).reshape(RT, 128, C)
        pr2 = np.ascontiguousarray(pr.transpose(1, 0, 2).reshape(
            128, RT * C))

        in_maps.append({
            "ft8": ft8, "fn8": fn8, "fl8": fl8, "fnl8": fnl8,
            "rrow": rrow_h, "pred2": pr2, "aux": auxh,
        })

    prep = dict(S0=S0, S1=S1, NP=NP, partners=partners, pcnt=pcnt,
                corder=corder, dq=dq, sim_true=sim_true, sim_q=sim_q,
                dir_map=dir_map, xdir=xdir_all, pred=pred, lab=lab)
    return in_maps, prep


def _combine(results, meta, prep):
    accs = np.stack([r["acc_out"] for r in results]).astype(np.float64)
    S0 = prep["S0"]
    dq, sim_true, sim_q = prep["dq"], prep["sim_true"], prep["sim_q"]
    NP = prep["NP"]
    Bf = float(B)

    # ---- contrastive ----
    pos_pair = sum(
        -np.log(np.exp(np.asarray(sim_true[a]) / TEMPERATURE) + 1e-8).sum()
        for a in range(B) if sim_true[a] is not None)
    pos_sum = (pos_pair
               + B * (-np.log(np.exp(1.0 / TEMPERATURE) + 1e-8))
               + (Bf * Bf - B - NP) * (-np.log1p(1e-8)))

    M = (accs[:, :, meta["C_SIM"]:meta["C_SIM"] + 2].sum()
         + (C_MARGIN * HB * 2 * 128 * N_CORES
            - accs[:, :, meta["C_SIM"] + 2:meta["C_SIM"] + 4].sum()))
    pair_min = sum(np.minimum(np.asarray(sim_q[a]), C_MARGIN).sum()
                   for a in range(B) if sim_q[a] is not None)
    sum_min_diff = M - C_MARGIN * B - pair_min
    neg_sum = 0.5 * Bf * Bf - sum_min_diff
    lc = (pos_sum + neg_sum) / (Bf * Bf)

    # ---- triplet self term ----
    selfsum = (accs[:, :, meta["C_SELF"]:meta["C_SELF"] + RT].sum()
               + accs[:, :, meta["C_SELF2"]].sum())
    mp = sum(np.minimum(dq[a], T_MARGIN).sum()
             for a in range(B) if dq[a] is not None)
    n_diff_sum = Bf * Bf - (B + NP)
    self_part = n_diff_sum * T_MARGIN - (selfsum - B * T_MARGIN - mp)

    # ---- triplet pair term ----
    pair_part = 0.0
    for c in range(N_CORES):
        acc_c = accs[c]
        for (p, m, s, a) in prep["dir_map"][c]:
            x = prep["xdir"][c, p, (0 if m == 0 else S0) + s]
            S_ap = acc_c[p, meta["C_DIR"] + (0 if m == 0 else S0) + s]
            if m == 0 and s == 0:
                S_ap += acc_c[p, meta["C_SELF2"] + 1]
            corr = np.maximum(x - dq[a], 0.0).sum()
            pair_part += Bf * x - S_ap - corr
    lt = (self_part + pair_part) / (Bf + 1e-8)

    # ---- focal + label smoothing (host tail) ----
    pred, lab = prep["pred"], prep["lab"]
    se = np.empty(B)
    for c in range(N_CORES):
        for m in range(RT):
            se[c * R + m * 128:c * R + (m + 1) * 128] = \
                accs[c, :, meta["C_SE"] + m]
    lse = np.log(se)
    ptgt = pred[np.arange(B), lab]
    spred = pred.sum(axis=1)
    ce = lse - ptgt
    pt = np.exp(-ce)
    lf = (ALPHA * (1.0 - pt) ** GAMMA * ce).mean()
    ls = (-(OFF * (spred - C * lse)
            + (1.0 - SMOOTHING - OFF) * (ptgt - lse))).mean()

    total = (W_CONTRASTIVE * lc + W_TRIPLET * lt
             + W_FOCAL * lf + W_LABEL_SMOOTH * ls)
    return np.array([lc, lt, lf, ls, total], dtype=np.float32)


def kernel(pred, target, features):
    in_maps, prep = _host_prep(pred, target, features)
    nc, meta = _build(prep["S0"], prep["S1"])
    res = run_bass_kernel_spmd(nc, in_maps, core_ids=list(range(N_CORES)))
    return _combine(res.results, meta, prep)


if __name__ == "__main__":
    import reference

    inputs = reference.setup_inputs()
    expected = np.asarray(reference.reference(**inputs))
    actual = kernel(**{k: np.asarray(v) for k, v in inputs.items()})
    err = np.abs(actual - expected) / np.maximum(np.abs(expected), 1e-12)
    print("expected:", expected)
    print("actual:  ", actual)
    print("rel err: ", err)


# revision 25
# speedup vs baseline: 1.0621x; 1.0068x over previous
"""Trainium2 Bass kernel v5 for nn_EnhancedLossModule.

Per-core plan (8 cores, 256 rows each, SPMD-uniform program):
  - Host precomputes fp8(e4m3) features Q, normalized Qn, exact row
    norms r of Q, and all same-label pair dot products (thresholds for
    the triplet pair reductions + contrastive pair corrections).
  - d2 = r_i + r_j - 2*Q_i.Q_j via fp8 DoubleRow matmuls (0.5 cyc/row)
    + one fp16 rank-1 matmul (ones x r_row) + a tiny one-hot matmul
    that adds 4096 to each row's own column (diag mask, NaN-safe sqrt).
    Columns are rotated by c*R per core so the diag block is always in
    the first 256 columns -> the program is identical on every core.
  - PSUM is used as [128, 1024] half-tiles (2 banks each, 4 in flight)
    so PE streams without bank stalls; warm-up matmuls ramp the PE
    p-state before the real work arrives.
  - ACT does sqrt(psum + r_i) -> dpt fp16 and the focal exp pass ->
    exactly 2 activation-table loads.
  - Triplet reductions: sum_n min(d', x) via DVE tensor_scalar
    min+accum passes: threshold 1.0 (self term) plus S0/S1 per-anchor
    threshold columns (rows sorted by partner count so the heavy
    anchors share tile m0).
  - sim = Qn_i.Qn_j via fp8 DoubleRow matmuls; sum_n min(sim, 0.5)
    reduced straight from PSUM halves (DVE).
  - Focal/label-smoothing: HW computes per-row sum(exp(pred_bf16));
    host does the O(B) log/pow tail.
"""

import os

import ml_dtypes
import numpy as np

import concourse.bacc as bacc
import concourse.bass as bass
import concourse.tile as tile
from concourse import mybir
from concourse.bass_utils import run_bass_kernel_spmd

B, C, D = 2048, 1000, 512
N_CORES = 8
R = B // N_CORES            # 256 rows per core
RT = R // 128               # 2 row tiles
KT = D // 128               # 4 contraction tiles (2 DoubleRow pairs)
HB = B // 2                 # psum half-tile width

TEMPERATURE = 0.07
C_MARGIN = 0.5
T_MARGIN = 1.0
GAMMA = 2.0
ALPHA = 0.25
SMOOTHING = 0.1
W_CONTRASTIVE = 0.1
W_TRIPLET = 0.1
W_FOCAL = 0.4
W_LABEL_SMOOTH = 0.4

DIAG = 4096.0               # added to d2 of each row's own column
OFF = SMOOTHING / (C - 1)
WARMN = int(os.environ.get("WARMN", "22"))
WARM2 = int(os.environ.get("WARM2", "2"))
EXPWAIT = float(os.environ.get("EXPWAIT_MS", "0.0095"))

F32 = mybir.dt.float32
BF16 = mybir.dt.bfloat16
FP16 = mybir.dt.float16
F8E4 = mybir.dt.float8e4
ALU = mybir.AluOpType
AF = mybir.ActivationFunctionType
E4M3 = ml_dtypes.float8_e4m3fn

_BUILD_CACHE: dict = {}


def _ap3(t, off, d1, n1, d2, n2):
    """3-dim AP view of a 2-D tile: [[pstride,128],[d1,n1],[d2,n2]]."""
    a = t[:, :]
    return bass.AP(tensor=a.tensor, offset=a.offset + off,
                   ap=[[a.ap[0][0], 128], [d1, n1], [d2, n2]])


def _build(S0: int, S1: int):
    key = (S0, S1)
    if key in _BUILD_CACHE:
        return _BUILD_CACHE[key]
    SD = S0 + S1

    # accumulator columns
    C_SELF = 0                  # RT: sum min(d', 1) per row tile
    C_DIR = C_SELF + RT         # SD: per-anchor pair sums
    C_SIM = C_DIR + SD          # 2*RT: sum min(sim, 0.5) per half
    C_SE = C_SIM + 2 * RT       # RT: sum exp(pred)
    C_SELF2 = C_SE + RT         # 2: h1 halves of split self/s0 passes
    NCOL = C_SELF2 + 2

    nc = bacc.Bacc("TRN2", target_bir_lowering=False, debug=False,
                   num_devices=N_CORES)

    ft8 = nc.dram_tensor("ft8", [128, KT * B], F8E4, kind="ExternalInput")
    fn8 = nc.dram_tensor("fn8", [128, KT * B], F8E4, kind="ExternalInput")
    fl8 = nc.dram_tensor("fl8", [128, KT * R], F8E4, kind="ExternalInput")
    fnl8 = nc.dram_tensor("fnl8", [128, KT * R], F8E4,
                          kind="ExternalInput")
    rrow = nc.dram_tensor("rrow", [1, B], FP16, kind="ExternalInput")
    pred2 = nc.dram_tensor("pred2", [128, RT * C], BF16,
                           kind="ExternalInput")
    # aux f32: [rloc RT][colx RT][xdir SD]
    NAUX = 2 * RT + SD
    aux = nc.dram_tensor("aux", [128, NAUX], F32, kind="ExternalInput")
    acc_out = nc.dram_tensor("acc_out", [128, NCOL], F32,
                             kind="ExternalOutput")
    A_RLOC, A_COLX, A_XDIR = 0, RT, 2 * RT

    with tile.TileContext(nc) as tc:
        with (
            tc.tile_pool(name="persist", bufs=1) as persist,
            tc.tile_pool(name="gpsum", bufs=4, space="PSUM") as gpsum,
        ):
            # ---------------- inputs ----------------
            iota256 = persist.tile([128, 256], F32)
            nc.gpsimd.iota(iota256, pattern=[[1, 256]], base=0,
                           channel_multiplier=0,
                           allow_small_or_imprecise_dtypes=True)
            pid = persist.tile([128, 1], F32)
            nc.gpsimd.iota(pid, pattern=[[0, 1]], base=0,
                           channel_multiplier=1,
                           allow_small_or_imprecise_dtypes=True)
            auxt = persist.tile([128, NAUX], F32)
            nc.scalar.dma_start(out=auxt, in_=aux.ap())
            rro = persist.tile([1, B], FP16)
            nc.scalar.dma_start(out=rro, in_=rrow.ap())
            fln = persist.tile([128, 2 * KT * R], F8E4)
            nc.gpsimd.dma_start(out=fln[:, :KT * R], in_=fl8.ap())
            nc.gpsimd.dma_start(out=fln[:, KT * R:], in_=fnl8.ap())
            fl = fln[:, :KT * R]
            fnl = fln[:, KT * R:]

            ft = [persist.tile([128, 2 * B], F8E4, name=f"ftk{kp}")
                  for kp in range(KT // 2)]
            fn = [persist.tile([128, 2 * B], F8E4, name=f"fnk{kp}")
                  for kp in range(KT // 2)]

            def qdma(dst, dsrc, kp, h):
                s = dsrc.ap()
                src_ap = bass.AP(
                    tensor=s.tensor, offset=s.offset + 2 * kp * B + h * HB,
                    ap=[[s.ap[0][0], 128], [B, 2], [1, HB]])
                d = dst[kp][:, :]
                dst_ap = bass.AP(
                    tensor=d.tensor, offset=d.offset + h * HB,
                    ap=[[d.ap[0][0], 128], [B, 2], [1, HB]])
                nc.sync.dma_start(out=dst_ap, in_=src_ap)

            qdma(ft, ft8, 0, 0)
            qdma(ft, ft8, 1, 0)
            qdma(ft, ft8, 0, 1)
            qdma(ft, ft8, 1, 1)
            qdma(fn, fn8, 0, 0)
            qdma(fn, fn8, 1, 0)
            qdma(fn, fn8, 0, 1)
            qdma(fn, fn8, 1, 1)
            pr2 = persist.tile([128, RT * C], BF16)
            nc.sync.dma_start(out=pr2, in_=pred2.ap())

            # ---------------- constants ----------------
            ident = persist.tile([128, 128], FP16)
            nc.vector.tensor_scalar(out=ident, in0=iota256[:, 0:128],
                                    scalar1=pid, scalar2=None,
                                    op0=ALU.is_equal)
            ones1 = persist.tile([1, 128], FP16)
            nc.vector.memset(ones1, 1.0)
            pm = persist.tile([128, RT * 256], FP16)
            for m in range(RT):
                nc.vector.tensor_scalar(
                    out=pm[:, m * 256:(m + 1) * 256], in0=iota256,
                    scalar1=auxt[:, A_COLX + m:A_COLX + m + 1],
                    scalar2=DIAG, op0=ALU.is_equal, op1=ALU.mult)
            halfc = persist.tile([128, 1], F32)
            nc.vector.memset(halfc, C_MARGIN)
            junk_v = persist.tile([128, B], FP16)
            junk_v2 = persist.tile([128, B], FP16)
            junk_w = persist.tile([128, HB], FP16)
            junk_a = persist.tile([128, C], BF16)
            acc = persist.tile([128, NCOL], F32)
            nc.vector.memset(acc, 0.0)
            dpt = [persist.tile([128, B], FP16, name=f"dpt{m}")
                   for m in range(RT)]
            # early dummy sqrt binds the sqrt-table load to idle time
            tiny = persist.tile([128, 1], FP16)
            nc.scalar.activation(out=tiny, in_=pid, func=AF.Sqrt)

            # ---------------- matmuls ----------------
            def mm_kp(ps, srck, m, h, kp, lo, start, stop):
                for ch in range(2):
                    o = ch * 512
                    nc.tensor.matmul(
                        ps[:, o:o + 512],
                        _ap3(fln, lo + 2 * kp * R + m * 128, R, 2, 1, 128),
                        _ap3(srck[kp], h * HB + o, B, 2, 1, 512),
                        start=start, stop=stop,
                        perf_mode=mybir.MatmulPerfMode.DoubleRow,
                        skip_group_check=True,
                    )

            def mm_finish(ps, m, h):
                if h == 0:
                    nc.tensor.matmul(
                        ps[:, 0:256], ident[:, :],
                        pm[:, m * 256:(m + 1) * 256],
                        start=False, stop=False, skip_group_check=True,
                    )
                for ch in range(2):
                    o = ch * 512
                    nc.tensor.matmul(
                        ps[:, o:o + 512], ones1[0:1, :],
                        rro[0:1, h * HB + o:h * HB + o + 512],
                        start=False, stop=True, skip_group_check=True,
                    )

            FLO = 0
            FNO = KT * R
            d2ps = [[gpsum.tile([128, HB], F32, tag="big",
                                name=f"d2ps{m}{h}") for h in range(2)]
                    for m in range(RT)]
            simps = [[None] * 2 for _ in range(RT)]

            for w in range(WARMN):
                nc.tensor.matmul(d2ps[0][1][:, 0:128], ident[:, :],
                                 ident[:, :], start=True, stop=True,
                                 skip_group_check=True)
            for m in range(RT):
                mm_kp(d2ps[m][0], ft, m, 0, 0, FLO, True, False)
            for w in range(WARM2):
                nc.tensor.matmul(d2ps[0][1][:, 0:128], ident[:, :],
                                 ident[:, :], start=True, stop=True,
                                 skip_group_check=True)
            for m in range(RT):
                mm_kp(d2ps[m][0], ft, m, 0, 1, FLO, False, False)
                mm_finish(d2ps[m][0], m, 0)
            for m in range(RT):
                mm_kp(d2ps[m][1], ft, m, 1, 0, FLO, True, False)
            for m in range(RT):
                mm_kp(d2ps[m][1], ft, m, 1, 1, FLO, False, False)
                mm_finish(d2ps[m][1], m, 1)
            for h in range(2):
                for m in range(RT):
                    simps[m][h] = gpsum.tile([128, HB], F32, tag="big",
                                             name=f"simps{m}{h}")
            for m in range(RT):
                mm_kp(simps[m][0], fn, m, 0, 0, FNO, True, False)
            for m in range(RT):
                mm_kp(simps[m][0], fn, m, 0, 1, FNO, False, True)
            for m in range(RT):
                mm_kp(simps[m][1], fn, m, 1, 0, FNO, True, False)
            for m in range(RT):
                mm_kp(simps[m][1], fn, m, 1, 1, FNO, False, True)

            # ---------------- ACT: sqrts, exps, relus ------------------
            for m in range(RT):
                for h in range(2):
                    nc.scalar.activation(
                        out=dpt[m][:, h * HB:(h + 1) * HB],
                        in_=d2ps[m][h], func=AF.Sqrt,
                        bias=auxt[:, A_RLOC + m:A_RLOC + m + 1])
            for m in range(RT):
                nc.scalar.activation(
                    out=junk_a, in_=pr2[:, m * C:(m + 1) * C],
                    func=AF.Exp,
                    accum_out=acc[:, C_SE + m:C_SE + m + 1])
            nc.scalar.activation(
                out=junk_w, in_=simps[1][0], func=AF.Relu,
                scale=-1.0, bias=halfc[:, 0:1],
                accum_out=acc[:, C_SIM + 1:C_SIM + 2])
            for m in range(RT):
                nc.scalar.activation(
                    out=junk_w, in_=simps[m][1], func=AF.Relu,
                    scale=-1.0, bias=halfc[:, 0:1],
                    accum_out=acc[:, C_SIM + 2 + m:C_SIM + 2 + m + 1])

            # ---------------- DVE: triplet + sim reductions ------------
            jv = [junk_v, junk_v2]
            ji = 0
            # first two m0 passes split by half so DVE starts on dpt0-h0
            nc.vector.tensor_scalar(
                out=junk_v[:, 0:HB], in0=dpt[0][:, 0:HB],
                scalar1=float(T_MARGIN), scalar2=None,
                op0=ALU.min, op1=ALU.add,
                accum_out=acc[:, C_SELF:C_SELF + 1])
            nc.vector.tensor_scalar(
                out=junk_v2[:, 0:HB], in0=dpt[0][:, 0:HB],
                scalar1=auxt[:, A_XDIR:A_XDIR + 1],
                scalar2=None, op0=ALU.min, op1=ALU.add,
                accum_out=acc[:, C_DIR:C_DIR + 1])
            nc.vector.tensor_scalar(
                out=jv[ji % 2][:, 0:HB], in0=simps[0][0],
                scalar1=float(C_MARGIN), scalar2=None,
                op0=ALU.min, op1=ALU.add,
                accum_out=acc[:, C_SIM:C_SIM + 1])
            ji += 1
            nc.vector.tensor_scalar(
                out=junk_v[:, 0:HB], in0=dpt[0][:, HB:B],
                scalar1=float(T_MARGIN), scalar2=None,
                op0=ALU.min, op1=ALU.add,
                accum_out=acc[:, C_SELF2:C_SELF2 + 1])
            nc.vector.tensor_scalar(
                out=junk_v2[:, 0:HB], in0=dpt[0][:, HB:B],
                scalar1=auxt[:, A_XDIR:A_XDIR + 1],
                scalar2=None, op0=ALU.min, op1=ALU.add,
                accum_out=acc[:, C_SELF2 + 1:C_SELF2 + 2])
            for m in range(RT):
                if m > 0:
                    nc.vector.tensor_scalar(
                        out=jv[ji % 2], in0=dpt[m],
                        scalar1=float(T_MARGIN),
                        scalar2=None, op0=ALU.min, op1=ALU.add,
                        accum_out=acc[:, C_SELF + m:C_SELF + m + 1])
                    ji += 1
                ns = S0 if m == 0 else S1
                off = 0 if m == 0 else S0
                for s in range(ns):
                    if m == 0 and s == 0:
                        continue
                    nc.vector.tensor_scalar(
                        out=jv[ji % 2], in0=dpt[m],
                        scalar1=auxt[:, A_XDIR + off + s:
                                     A_XDIR + off + s + 1],
                        scalar2=None, op0=ALU.min, op1=ALU.add,
                        accum_out=acc[:, C_DIR + off + s:
                                      C_DIR + off + s + 1])
                    ji += 1


            nc.sync.dma_start(out=acc_out.ap(), in_=acc)

    nc.compile()
    meta = dict(S0=S0, S1=S1, NCOL=NCOL, C_SELF=C_SELF, C_DIR=C_DIR,
                C_SIM=C_SIM, C_SE=C_SE, C_SELF2=C_SELF2)
    _BUILD_CACHE[key] = (nc, meta)
    return nc, meta


def _host_prep(pred, target, features):
    pred = np.asarray(pred, dtype=np.float64)
    lab = np.asarray(target).astype(np.int64)
    f = np.asarray(features, dtype=np.float64)

    Q = f.astype(np.float32).astype(E4M3)
    Qf = Q.astype(np.float64)
    r = np.einsum("ij,ij->i", Qf, Qf)                  # exact fp8 row norms
    rr16 = r.astype(np.float16)
    rr16f = rr16.astype(np.float64)

    nrm = np.linalg.norm(f, axis=1)
    n = f / nrm[:, None]
    Qn = n.astype(np.float32).astype(E4M3)
    Qnf = Qn.astype(np.float64)

    # same-label partner lists
    order = np.argsort(lab, kind="stable")
    sl = lab[order]
    starts = np.flatnonzero(np.r_[True, sl[1:] != sl[:-1]])
    ends = np.r_[starts[1:], len(sl)]
    partners = [[] for _ in range(B)]
    for s, e in zip(starts, ends):
        if e - s < 2:
            continue
        mem = order[s:e]
        for a in mem:
            for p in mem:
                if p != a:
                    partners[a].append(int(p))
    pcnt = np.array([len(p) for p in partners])
    NP = int(pcnt.sum())

    # in-core sort by partner count (heavy anchors into tile m0)
    corder = np.empty((N_CORES, R), np.int64)
    for c in range(N_CORES):
        seg = np.arange(c * R, (c + 1) * R)
        corder[c] = seg[np.argsort(-pcnt[seg], kind="stable")]

    S0 = max(1, int(max(pcnt[corder[c][:128]].max()
                        for c in range(N_CORES))))
    S1 = max(1, int(max(pcnt[corder[c][128:]].max()
                        for c in range(N_CORES))))

    # pair values (Q-space distances; original/Qn-space sims)
    dq = [None] * B
    sim_true = [None] * B
    sim_q = [None] * B
    for a in range(B):
        ps = partners[a]
        if not ps:
            continue
        P = np.array(ps)
        g = Qf[P] @ Qf[a]
        d2 = r[a] + rr16f[P] - 2.0 * g
        dq[a] = np.sqrt(np.maximum(d2, 0.0))
        sim_true[a] = (f[P] @ f[a]) / (nrm[a] * nrm[P])
        sim_q[a] = Qnf[P] @ Qnf[a]

    SD = S0 + S1
    in_maps = []
    xdir_all = np.zeros((N_CORES, 128, SD), np.float64)
    dir_map = [[] for _ in range(N_CORES)]  # (p, m, s, anchor)

    ftT = np.ascontiguousarray(Q.T)          # [D, B]
    fnT = np.ascontiguousarray(Qn.T)

    for c in range(N_CORES):
        rot = np.roll(np.arange(B), -c * R)  # rotated column order
        ft8 = np.empty((128, KT * B), E4M3)
        fn8 = np.empty((128, KT * B), E4M3)
        for k in range(KT):
            ft8[:, k * B:(k + 1) * B] = ftT[k * 128:(k + 1) * 128, rot]
            fn8[:, k * B:(k + 1) * B] = fnT[k * 128:(k + 1) * 128, rot]
        rows = corder[c]
        fl8 = np.empty((128, KT * R), E4M3)
        fnl8 = np.empty((128, KT * R), E4M3)
        m2q = (-2.0 * Qf[rows]).astype(E4M3)      # exact in e4m3
        for k in range(KT):
            fl8[:, k * R:(k + 1) * R] = m2q[:, k * 128:(k + 1) * 128].T
            fnl8[:, k * R:(k + 1) * R] = \
                fnT[k * 128:(k + 1) * 128][:, rows]
        rrow_h = rr16[rot][None, :]

        auxh = np.zeros((128, 2 * RT + SD), np.float32)
        for m in range(RT):
            Sm = S0 if m == 0 else S1
            soff = 0 if m == 0 else S0
            for p in range(128):
                g = int(rows[m * 128 + p])
                auxh[p, m] = r[g]                     # rloc (fp32 bias)
                auxh[p, RT + m] = (g - c * R) % B     # colx (rotated)
                ps = partners[g]
                for s in range(min(len(ps), Sm)):
                    x = dq[g][s] + T_MARGIN
                    auxh[p, 2 * RT + soff + s] = x
                    xdir_all[c, p, soff + s] = x
                    dir_map[c].append((p, m, s, g))

        pr = np.asarray(pred, np.float32)[c * R:(c + 1) * R]
        pr = pr.astype(ml_dtypes.bfloat16).reshape(RT, 128, C)
        pr2 = np.ascontiguousarray(pr.transpose(1, 0, 2).reshape(
            128, RT * C))

        in_maps.append({
            "ft8": ft8, "fn8": fn8, "fl8": fl8, "fnl8": fnl8,
            "rrow": rrow_h, "pred2": pr2, "aux": auxh,
        })

    prep = dict(S0=S0, S1=S1, NP=NP, partners=partners, pcnt=pcnt,
                corder=corder, dq=dq, sim_true=sim_true, sim_q=sim_q,
                dir_map=dir_map, xdir=xdir_all, pred=pred, lab=lab)
    return in_maps, prep


def _combine(results, meta, prep):
    accs = np.stack([r["acc_out"] for r in results]).astype(np.float64)
    S0 = prep["S0"]
    dq, sim_true, sim_q = prep["dq"], prep["sim_true"], prep["sim_q"]
    NP = prep["NP"]
    Bf = float(B)

    # ---- contrastive ----
    pos_pair = sum(
        -np.log(np.exp(np.asarray(sim_true[a]) / TEMPERATURE) + 1e-8).sum()
        for a in range(B) if sim_true[a] is not None)
    pos_sum = (pos_pair
               + B * (-np.log(np.exp(1.0 / TEMPERATURE) + 1e-8))
               + (Bf * Bf - B - NP) * (-np.log1p(1e-8)))

    M = (accs[:, :, meta["C_SIM"]].sum()
         + (C_MARGIN * HB * 3 * 128 * N_CORES
            - accs[:, :, meta["C_SIM"] + 1:meta["C_SIM"] + 4].sum()))
    pair_min = sum(np.minimum(np.asarray(sim_q[a]), C_MARGIN).sum()
                   for a in range(B) if sim_q[a] is not None)
    sum_min_diff = M - C_MARGIN * B - pair_min
    neg_sum = 0.5 * Bf * Bf - sum_min_diff
    lc = (pos_sum + neg_sum) / (Bf * Bf)

    # ---- triplet self term ----
    selfsum = (accs[:, :, meta["C_SELF"]:meta["C_SELF"] + RT].sum()
               + accs[:, :, meta["C_SELF2"]].sum())
    mp = sum(np.minimum(dq[a], T_MARGIN).sum()
             for a in range(B) if dq[a] is not None)
    n_diff_sum = Bf * Bf - (B + NP)
    self_part = n_diff_sum * T_MARGIN - (selfsum - B * T_MARGIN - mp)

    # ---- triplet pair term ----
    pair_part = 0.0
    for c in range(N_CORES):
        acc_c = accs[c]
        for (p, m, s, a) in prep["dir_map"][c]:
            x = prep["xdir"][c, p, (0 if m == 0 else S0) + s]
            S_ap = acc_c[p, meta["C_DIR"] + (0 if m == 0 else S0) + s]
            if m == 0 and s == 0:
                S_ap += acc_c[p, meta["C_SELF2"] + 1]
            corr = np.maximum(x - dq[a], 0.0).sum()
            pair_part += Bf * x - S_ap - corr
    lt = (self_part + pair_part) / (Bf + 1e-8)

    # ---- focal + label smoothing (host tail) ----
    pred, lab = prep["pred"], prep["lab"]
    se = np.empty(B)
    for c in range(N_CORES):
        for m in range(RT):
            se[c * R + m * 128:c * R + (m + 1) * 128] = \
                accs[c, :, meta["C_SE"] + m]
    lse = np.log(se)
    ptgt = pred[np.arange(B), lab]
    spred = pred.sum(axis=1)
    ce = lse - ptgt
    pt = np.exp(-ce)
    lf = (ALPHA * (1.0 - pt) ** GAMMA * ce).mean()
    ls = (-(OFF * (spred - C * lse)
            + (1.0 - SMOOTHING - OFF) * (ptgt - lse))).mean()

    total = (W_CONTRASTIVE * lc + W_TRIPLET * lt
             + W_FOCAL * lf + W_LABEL_SMOOTH * ls)
    return np.array([lc, lt, lf, ls, total], dtype=np.float32)


def kernel(pred, target, features):
    in_maps, prep = _host_prep(pred, target, features)
    nc, meta = _build(prep["S0"], prep["S1"])
    res = run_bass_kernel_spmd(nc, in_maps, core_ids=list(range(N_CORES)))
    return _combine(res.results, meta, prep)


if __name__ == "__main__":
    import reference

    inputs = reference.setup_inputs()
    expected = np.asarray(reference.reference(**inputs))
    actual = kernel(**{k: np.asarray(v) for k, v in inputs.items()})
    err = np.abs(actual - expected) / np.maximum(np.abs(expected), 1e-12)
    print("expected:", expected)
    print("actual:  ", actual)
    print("rel err: ", err)


# revision 28
# speedup vs baseline: 1.0829x; 1.0196x over previous
"""Trainium2 Bass kernel v5 for nn_EnhancedLossModule.

Per-core plan (8 cores, 256 rows each, SPMD-uniform program):
  - Host precomputes fp8(e4m3) features Q, normalized Qn, exact row
    norms r of Q, and all same-label pair dot products (thresholds for
    the triplet pair reductions + contrastive pair corrections).
  - d2 = r_i + r_j - 2*Q_i.Q_j via fp8 DoubleRow matmuls (0.5 cyc/row)
    + one fp16 rank-1 matmul (ones x r_row) + a tiny one-hot matmul
    that adds 4096 to each row's own column (diag mask, NaN-safe sqrt).
    Columns are rotated by c*R per core so the diag block is always in
    the first 256 columns -> the program is identical on every core.
  - PSUM is used as [128, 1024] half-tiles (2 banks each, 4 in flight)
    so PE streams without bank stalls; warm-up matmuls ramp the PE
    p-state before the real work arrives.
  - ACT does sqrt(psum + r_i) -> dpt fp16 and the focal exp pass ->
    exactly 2 activation-table loads.
  - Triplet reductions: sum_n min(d', x) via DVE tensor_scalar
    min+accum passes: threshold 1.0 (self term) plus S0/S1 per-anchor
    threshold columns (rows sorted by partner count so the heavy
    anchors share tile m0).
  - sim = Qn_i.Qn_j via fp8 DoubleRow matmuls; sum_n min(sim, 0.5)
    reduced straight from PSUM halves (DVE).
  - Focal/label-smoothing: HW computes per-row sum(exp(pred_bf16));
    host does the O(B) log/pow tail.
"""

import os

import ml_dtypes
import numpy as np

import concourse.bacc as bacc
import concourse.bass as bass
import concourse.tile as tile
from concourse import mybir
from concourse.bass_utils import run_bass_kernel_spmd

B, C, D = 2048, 1000, 512
N_CORES = 8
R = B // N_CORES            # 256 rows per core
RT = R // 128               # 2 row tiles
KT = D // 128               # 4 contraction tiles (2 DoubleRow pairs)
HB = B // 2                 # psum half-tile width

TEMPERATURE = 0.07
C_MARGIN = 0.5
T_MARGIN = 1.0
GAMMA = 2.0
ALPHA = 0.25
SMOOTHING = 0.1
W_CONTRASTIVE = 0.1
W_TRIPLET = 0.1
W_FOCAL = 0.4
W_LABEL_SMOOTH = 0.4

DIAG = 4096.0               # added to d2 of each row's own column
OFF = SMOOTHING / (C - 1)
WARMN = int(os.environ.get("WARMN", "22"))
WARM2 = int(os.environ.get("WARM2", "2"))
EXPWAIT = float(os.environ.get("EXPWAIT_MS", "0.0095"))

F32 = mybir.dt.float32
BF16 = mybir.dt.bfloat16
FP16 = mybir.dt.float16
F8E4 = mybir.dt.float8e4
ALU = mybir.AluOpType
AF = mybir.ActivationFunctionType
E4M3 = ml_dtypes.float8_e4m3fn

_BUILD_CACHE: dict = {}


def _ap3(t, off, d1, n1, d2, n2):
    """3-dim AP view of a 2-D tile: [[pstride,128],[d1,n1],[d2,n2]]."""
    a = t[:, :]
    return bass.AP(tensor=a.tensor, offset=a.offset + off,
                   ap=[[a.ap[0][0], 128], [d1, n1], [d2, n2]])


def _build(S0: int, S1: int):
    key = (S0, S1)
    if key in _BUILD_CACHE:
        return _BUILD_CACHE[key]
    SD = S0 + S1

    # accumulator columns
    C_SELF = 0                  # RT: sum min(d', 1) per row tile
    C_DIR = C_SELF + RT         # SD: per-anchor pair sums
    C_SIM = C_DIR + SD          # 2*RT: sum min(sim, 0.5) per half
    C_SE = C_SIM + 2 * RT       # RT: sum exp(pred)
    C_SELF2 = C_SE + RT         # 2: h1 halves of split self/s0 passes
    C_DIR2 = C_SELF2 + 2        # 2: h1 halves of split s1/s2 passes
    NCOL = C_DIR2 + 2

    nc = bacc.Bacc("TRN2", target_bir_lowering=False, debug=False,
                   num_devices=N_CORES)

    ft8 = nc.dram_tensor("ft8", [128, KT * B], F8E4, kind="ExternalInput")
    fn8 = nc.dram_tensor("fn8", [128, KT * B], F8E4, kind="ExternalInput")
    fl8 = nc.dram_tensor("fl8", [128, KT * R], F8E4, kind="ExternalInput")
    fnl8 = nc.dram_tensor("fnl8", [128, KT * R], F8E4,
                          kind="ExternalInput")
    rrow = nc.dram_tensor("rrow", [1, B], FP16, kind="ExternalInput")
    pred2 = nc.dram_tensor("pred2", [128, RT * C], BF16,
                           kind="ExternalInput")
    # aux f32: [rloc RT][colx RT][xdir SD]
    NAUX = 2 * RT + SD
    aux = nc.dram_tensor("aux", [128, NAUX], F32, kind="ExternalInput")
    acc_out = nc.dram_tensor("acc_out", [128, NCOL], F32,
                             kind="ExternalOutput")
    A_RLOC, A_COLX, A_XDIR = 0, RT, 2 * RT

    with tile.TileContext(nc) as tc:
        with (
            tc.tile_pool(name="persist", bufs=1) as persist,
            tc.tile_pool(name="gpsum", bufs=4, space="PSUM") as gpsum,
        ):
            # ---------------- inputs ----------------
            iota256 = persist.tile([128, 256], F32)
            nc.gpsimd.iota(iota256, pattern=[[1, 256]], base=0,
                           channel_multiplier=0,
                           allow_small_or_imprecise_dtypes=True)
            pid = persist.tile([128, 1], F32)
            nc.gpsimd.iota(pid, pattern=[[0, 1]], base=0,
                           channel_multiplier=1,
                           allow_small_or_imprecise_dtypes=True)
            auxt = persist.tile([128, NAUX], F32)
            nc.scalar.dma_start(out=auxt, in_=aux.ap())
            rro = persist.tile([1, B], FP16)
            nc.scalar.dma_start(out=rro, in_=rrow.ap())
            fln = persist.tile([128, 2 * KT * R], F8E4)
            nc.gpsimd.dma_start(out=fln[:, :KT * R], in_=fl8.ap())
            nc.gpsimd.dma_start(out=fln[:, KT * R:], in_=fnl8.ap())
            fl = fln[:, :KT * R]
            fnl = fln[:, KT * R:]

            ft = [persist.tile([128, 2 * B], F8E4, name=f"ftk{kp}")
                  for kp in range(KT // 2)]
            fn = [persist.tile([128, 2 * B], F8E4, name=f"fnk{kp}")
                  for kp in range(KT // 2)]

            def qdma(dst, dsrc, kp, h):
                s = dsrc.ap()
                src_ap = bass.AP(
                    tensor=s.tensor, offset=s.offset + 2 * kp * B + h * HB,
                    ap=[[s.ap[0][0], 128], [B, 2], [1, HB]])
                d = dst[kp][:, :]
                dst_ap = bass.AP(
                    tensor=d.tensor, offset=d.offset + h * HB,
                    ap=[[d.ap[0][0], 128], [B, 2], [1, HB]])
                nc.sync.dma_start(out=dst_ap, in_=src_ap)

            qdma(ft, ft8, 0, 0)
            qdma(ft, ft8, 1, 0)
            qdma(ft, ft8, 0, 1)
            qdma(ft, ft8, 1, 1)
            qdma(fn, fn8, 0, 0)
            qdma(fn, fn8, 1, 0)
            qdma(fn, fn8, 0, 1)
            qdma(fn, fn8, 1, 1)
            pr2 = persist.tile([128, RT * C], BF16)
            nc.sync.dma_start(out=pr2, in_=pred2.ap())

            # ---------------- constants ----------------
            ident = persist.tile([128, 128], FP16)
            nc.vector.tensor_scalar(out=ident, in0=iota256[:, 0:128],
                                    scalar1=pid, scalar2=None,
                                    op0=ALU.is_equal)
            ones1 = persist.tile([1, 128], FP16)
            nc.vector.memset(ones1, 1.0)
            pm = persist.tile([128, RT * 256], FP16)
            for m in range(RT):
                nc.vector.tensor_scalar(
                    out=pm[:, m * 256:(m + 1) * 256], in0=iota256,
                    scalar1=auxt[:, A_COLX + m:A_COLX + m + 1],
                    scalar2=DIAG, op0=ALU.is_equal, op1=ALU.mult)
            halfc = persist.tile([128, 1], F32)
            nc.vector.memset(halfc, C_MARGIN)
            junk_v = persist.tile([128, B], FP16)
            junk_v2 = persist.tile([128, B], FP16)
            junk_w = persist.tile([128, HB], FP16)
            junk_a = persist.tile([128, C], BF16)
            acc = persist.tile([128, NCOL], F32)
            nc.vector.memset(acc, 0.0)
            dpt = [persist.tile([128, B], FP16, name=f"dpt{m}")
                   for m in range(RT)]
            # early dummy sqrt binds the sqrt-table load to idle time
            tiny = persist.tile([128, 1], FP16)
            nc.scalar.activation(out=tiny, in_=pid, func=AF.Sqrt)

            # ---------------- matmuls ----------------
            def mm_kp(ps, srck, m, h, kp, lo, start, stop):
                for ch in range(2):
                    o = ch * 512
                    nc.tensor.matmul(
                        ps[:, o:o + 512],
                        _ap3(fln, lo + 2 * kp * R + m * 128, R, 2, 1, 128),
                        _ap3(srck[kp], h * HB + o, B, 2, 1, 512),
                        start=start, stop=stop,
                        perf_mode=mybir.MatmulPerfMode.DoubleRow,
                        skip_group_check=True,
                    )

            def mm_finish(ps, m, h):
                if h == 0:
                    nc.tensor.matmul(
                        ps[:, 0:256], ident[:, :],
                        pm[:, m * 256:(m + 1) * 256],
                        start=False, stop=False, skip_group_check=True,
                    )
                for ch in range(2):
                    o = ch * 512
                    nc.tensor.matmul(
                        ps[:, o:o + 512], ones1[0:1, :],
                        rro[0:1, h * HB + o:h * HB + o + 512],
                        start=False, stop=True, skip_group_check=True,
                    )

            FLO = 0
            FNO = KT * R
            d2ps = [[gpsum.tile([128, HB], F32, tag="big",
                                name=f"d2ps{m}{h}") for h in range(2)]
                    for m in range(RT)]
            simps = [[None] * 2 for _ in range(RT)]

            for w in range(WARMN):
                nc.tensor.matmul(d2ps[0][1][:, 0:128], ident[:, :],
                                 ident[:, :], start=True, stop=True,
                                 skip_group_check=True)
            for m in range(RT):
                mm_kp(d2ps[m][0], ft, m, 0, 0, FLO, True, False)
            for w in range(WARM2):
                nc.tensor.matmul(d2ps[0][1][:, 0:128], ident[:, :],
                                 ident[:, :], start=True, stop=True,
                                 skip_group_check=True)
            for m in range(RT):
                mm_kp(d2ps[m][0], ft, m, 0, 1, FLO, False, False)
                mm_finish(d2ps[m][0], m, 0)
            for m in range(RT):
                mm_kp(d2ps[m][1], ft, m, 1, 0, FLO, True, False)
            for m in range(RT):
                mm_kp(d2ps[m][1], ft, m, 1, 1, FLO, False, False)
                mm_finish(d2ps[m][1], m, 1)
            for h in range(2):
                for m in range(RT):
                    simps[m][h] = gpsum.tile([128, HB], F32, tag="big",
                                             name=f"simps{m}{h}")
            for m in range(RT):
                mm_kp(simps[m][0], fn, m, 0, 0, FNO, True, False)
            for m in range(RT):
                mm_kp(simps[m][0], fn, m, 0, 1, FNO, False, True)
            for m in range(RT):
                mm_kp(simps[m][1], fn, m, 1, 0, FNO, True, False)
            for m in range(RT):
                mm_kp(simps[m][1], fn, m, 1, 1, FNO, False, True)

            # ---------------- ACT: sqrts, exps, relus ------------------
            for m in range(RT):
                for h in range(2):
                    nc.scalar.activation(
                        out=dpt[m][:, h * HB:(h + 1) * HB],
                        in_=d2ps[m][h], func=AF.Sqrt,
                        bias=auxt[:, A_RLOC + m:A_RLOC + m + 1])
            for m in range(RT):
                nc.scalar.activation(
                    out=junk_a, in_=pr2[:, m * C:(m + 1) * C],
                    func=AF.Exp,
                    accum_out=acc[:, C_SE + m:C_SE + m + 1])
            nc.scalar.activation(
                out=junk_w, in_=simps[1][0], func=AF.Relu,
                scale=-1.0, bias=halfc[:, 0:1],
                accum_out=acc[:, C_SIM + 1:C_SIM + 2])
            nc.scalar.activation(
                out=junk_w, in_=simps[0][1], func=AF.Relu,
                scale=-1.0, bias=halfc[:, 0:1],
                accum_out=acc[:, C_SIM + 2:C_SIM + 3])

            # ---------------- DVE: triplet + sim reductions ------------
            jv = [junk_v, junk_v2]
            ji = 0
            # first two m0 passes split by half so DVE starts on dpt0-h0
            nc.vector.tensor_scalar(
                out=junk_v[:, 0:HB], in0=dpt[0][:, 0:HB],
                scalar1=float(T_MARGIN), scalar2=None,
                op0=ALU.min, op1=ALU.add,
                accum_out=acc[:, C_SELF:C_SELF + 1])
            nc.vector.tensor_scalar(
                out=junk_v2[:, 0:HB], in0=dpt[0][:, 0:HB],
                scalar1=auxt[:, A_XDIR:A_XDIR + 1],
                scalar2=None, op0=ALU.min, op1=ALU.add,
                accum_out=acc[:, C_DIR:C_DIR + 1])
            for s in (1, 2):
                nc.vector.tensor_scalar(
                    out=junk_v[:, 0:HB], in0=dpt[0][:, 0:HB],
                    scalar1=auxt[:, A_XDIR + s:A_XDIR + s + 1],
                    scalar2=None, op0=ALU.min, op1=ALU.add,
                    accum_out=acc[:, C_DIR + s:C_DIR + s + 1])
            nc.vector.tensor_scalar(
                out=jv[ji % 2][:, 0:HB], in0=simps[0][0],
                scalar1=float(C_MARGIN), scalar2=None,
                op0=ALU.min, op1=ALU.add,
                accum_out=acc[:, C_SIM:C_SIM + 1])
            ji += 1
            nc.vector.tensor_scalar(
                out=junk_v[:, 0:HB], in0=dpt[0][:, HB:B],
                scalar1=float(T_MARGIN), scalar2=None,
                op0=ALU.min, op1=ALU.add,
                accum_out=acc[:, C_SELF2:C_SELF2 + 1])
            nc.vector.tensor_scalar(
                out=junk_v2[:, 0:HB], in0=dpt[0][:, HB:B],
                scalar1=auxt[:, A_XDIR:A_XDIR + 1],
                scalar2=None, op0=ALU.min, op1=ALU.add,
                accum_out=acc[:, C_SELF2 + 1:C_SELF2 + 2])
            for s in (1, 2):
                nc.vector.tensor_scalar(
                    out=junk_v[:, 0:HB], in0=dpt[0][:, HB:B],
                    scalar1=auxt[:, A_XDIR + s:A_XDIR + s + 1],
                    scalar2=None, op0=ALU.min, op1=ALU.add,
                    accum_out=acc[:, C_DIR2 + s - 1:C_DIR2 + s])
            nc.vector.tensor_scalar(
                out=junk_v2[:, 0:HB], in0=simps[1][1],
                scalar1=float(C_MARGIN), scalar2=None,
                op0=ALU.min, op1=ALU.add,
                accum_out=acc[:, C_SIM + 3:C_SIM + 4])
            for m in range(RT):
                if m > 0:
                    nc.vector.tensor_scalar(
                        out=jv[ji % 2], in0=dpt[m],
                        scalar1=float(T_MARGIN),
                        scalar2=None, op0=ALU.min, op1=ALU.add,
                        accum_out=acc[:, C_SELF + m:C_SELF + m + 1])
                    ji += 1
                ns = S0 if m == 0 else S1
                off = 0 if m == 0 else S0
                for s in range(ns):
                    if m == 0 and s <= 2:
                        continue
                    nc.vector.tensor_scalar(
                        out=jv[ji % 2], in0=dpt[m],
                        scalar1=auxt[:, A_XDIR + off + s:
                                     A_XDIR + off + s + 1],
                        scalar2=None, op0=ALU.min, op1=ALU.add,
                        accum_out=acc[:, C_DIR + off + s:
                                      C_DIR + off + s + 1])
                    ji += 1


            nc.sync.dma_start(out=acc_out.ap(), in_=acc)

    nc.compile()
    meta = dict(S0=S0, S1=S1, NCOL=NCOL, C_SELF=C_SELF, C_DIR=C_DIR,
                C_SIM=C_SIM, C_SE=C_SE, C_SELF2=C_SELF2, C_DIR2=C_DIR2)
    _BUILD_CACHE[key] = (nc, meta)
    return nc, meta


def _host_prep(pred, target, features):
    pred = np.asarray(pred, dtype=np.float64)
    lab = np.asarray(target).astype(np.int64)
    f = np.asarray(features, dtype=np.float64)

    Q = f.astype(np.float32).astype(E4M3)
    Qf = Q.astype(np.float64)
    r = np.einsum("ij,ij->i", Qf, Qf)                  # exact fp8 row norms
    rr16 = r.astype(np.float16)
    rr16f = rr16.astype(np.float64)

    nrm = np.linalg.norm(f, axis=1)
    n = f / nrm[:, None]
    Qn = n.astype(np.float32).astype(E4M3)
    Qnf = Qn.astype(np.float64)

    # same-label partner lists
    order = np.argsort(lab, kind="stable")
    sl = lab[order]
    starts = np.flatnonzero(np.r_[True, sl[1:] != sl[:-1]])
    ends = np.r_[starts[1:], len(sl)]
    partners = [[] for _ in range(B)]
    for s, e in zip(starts, ends):
        if e - s < 2:
            continue
        mem = order[s:e]
        for a in mem:
            for p in mem:
                if p != a:
                    partners[a].append(int(p))
    pcnt = np.array([len(p) for p in partners])
    NP = int(pcnt.sum())

    # in-core sort by partner count (heavy anchors into tile m0)
    corder = np.empty((N_CORES, R), np.int64)
    for c in range(N_CORES):
        seg = np.arange(c * R, (c + 1) * R)
        corder[c] = seg[np.argsort(-pcnt[seg], kind="stable")]

    S0 = max(1, int(max(pcnt[corder[c][:128]].max()
                        for c in range(N_CORES))))
    S1 = max(1, int(max(pcnt[corder[c][128:]].max()
                        for c in range(N_CORES))))

    # pair values (Q-space distances; original/Qn-space sims)
    dq = [None] * B
    sim_true = [None] * B
    sim_q = [None] * B
    for a in range(B):
        ps = partners[a]
        if not ps:
            continue
        P = np.array(ps)
        g = Qf[P] @ Qf[a]
        d2 = r[a] + rr16f[P] - 2.0 * g
        dq[a] = np.sqrt(np.maximum(d2, 0.0))
        sim_true[a] = (f[P] @ f[a]) / (nrm[a] * nrm[P])
        sim_q[a] = Qnf[P] @ Qnf[a]

    SD = S0 + S1
    in_maps = []
    xdir_all = np.zeros((N_CORES, 128, SD), np.float64)
    dir_map = [[] for _ in range(N_CORES)]  # (p, m, s, anchor)

    ftT = np.ascontiguousarray(Q.T)          # [D, B]
    fnT = np.ascontiguousarray(Qn.T)

    for c in range(N_CORES):
        rot = np.roll(np.arange(B), -c * R)  # rotated column order
        ft8 = np.empty((128, KT * B), E4M3)
        fn8 = np.empty((128, KT * B), E4M3)
        for k in range(KT):
            ft8[:, k * B:(k + 1) * B] = ftT[k * 128:(k + 1) * 128, rot]
            fn8[:, k * B:(k + 1) * B] = fnT[k * 128:(k + 1) * 128, rot]
        rows = corder[c]
        fl8 = np.empty((128, KT * R), E4M3)
        fnl8 = np.empty((128, KT * R), E4M3)
        m2q = (-2.0 * Qf[rows]).astype(E4M3)      # exact in e4m3
        for k in range(KT):
            fl8[:, k * R:(k + 1) * R] = m2q[:, k * 128:(k + 1) * 128].T
            fnl8[:, k * R:(k + 1) * R] = \
                fnT[k * 128:(k + 1) * 128][:, rows]
        rrow_h = rr16[rot][None, :]

        auxh = np.zeros((128, 2 * RT + SD), np.float32)
        for m in range(RT):
            Sm = S0 if m == 0 else S1
            soff = 0 if m == 0 else S0
            for p in range(128):
                g = int(rows[m * 128 + p])
                auxh[p, m] = r[g]                     # rloc (fp32 bias)
                auxh[p, RT + m] = (g - c * R) % B     # colx (rotated)
                ps = partners[g]
                for s in range(min(len(ps), Sm)):
                    x = dq[g][s] + T_MARGIN
                    auxh[p, 2 * RT + soff + s] = x
                    xdir_all[c, p, soff + s] = x
                    dir_map[c].append((p, m, s, g))

        pr = np.asarray(pred, np.float32)[c * R:(c + 1) * R]
        pr = pr.astype(ml_dtypes.bfloat16).reshape(RT, 128, C)
        pr2 = np.ascontiguousarray(pr.transpose(1, 0, 2).reshape(
            128, RT * C))

        in_maps.append({
            "ft8": ft8, "fn8": fn8, "fl8": fl8, "fnl8": fnl8,
            "rrow": rrow_h, "pred2": pr2, "aux": auxh,
        })

    prep = dict(S0=S0, S1=S1, NP=NP, partners=partners, pcnt=pcnt,
                corder=corder, dq=dq, sim_true=sim_true, sim_q=sim_q,
                dir_map=dir_map, xdir=xdir_all, pred=pred, lab=lab)
    return in_maps, prep


def _combine(results, meta, prep):
    accs = np.stack([r["acc_out"] for r in results]).astype(np.float64)
    S0 = prep["S0"]
    dq, sim_true, sim_q = prep["dq"], prep["sim_true"], prep["sim_q"]
    NP = prep["NP"]
    Bf = float(B)

    # ---- contrastive ----
    pos_pair = sum(
        -np.log(np.exp(np.asarray(sim_true[a]) / TEMPERATURE) + 1e-8).sum()
        for a in range(B) if sim_true[a] is not None)
    pos_sum = (pos_pair
               + B * (-np.log(np.exp(1.0 / TEMPERATURE) + 1e-8))
               + (Bf * Bf - B - NP) * (-np.log1p(1e-8)))

    M = (accs[:, :, meta["C_SIM"]].sum()
         + accs[:, :, meta["C_SIM"] + 3].sum()
         + (C_MARGIN * HB * 2 * 128 * N_CORES
            - accs[:, :, meta["C_SIM"] + 1:meta["C_SIM"] + 3].sum()))
    pair_min = sum(np.minimum(np.asarray(sim_q[a]), C_MARGIN).sum()
                   for a in range(B) if sim_q[a] is not None)
    sum_min_diff = M - C_MARGIN * B - pair_min
    neg_sum = 0.5 * Bf * Bf - sum_min_diff
    lc = (pos_sum + neg_sum) / (Bf * Bf)

    # ---- triplet self term ----
    selfsum = (accs[:, :, meta["C_SELF"]:meta["C_SELF"] + RT].sum()
               + accs[:, :, meta["C_SELF2"]].sum())
    mp = sum(np.minimum(dq[a], T_MARGIN).sum()
             for a in range(B) if dq[a] is not None)
    n_diff_sum = Bf * Bf - (B + NP)
    self_part = n_diff_sum * T_MARGIN - (selfsum - B * T_MARGIN - mp)

    # ---- triplet pair term ----
    pair_part = 0.0
    for c in range(N_CORES):
        acc_c = accs[c]
        for (p, m, s, a) in prep["dir_map"][c]:
            x = prep["xdir"][c, p, (0 if m == 0 else S0) + s]
            S_ap = acc_c[p, meta["C_DIR"] + (0 if m == 0 else S0) + s]
            if m == 0 and s == 0:
                S_ap += acc_c[p, meta["C_SELF2"] + 1]
            elif m == 0 and s in (1, 2):
                S_ap += acc_c[p, meta["C_DIR2"] + s - 1]
            corr = np.maximum(x - dq[a], 0.0).sum()
            pair_part += Bf * x - S_ap - corr
    lt = (self_part + pair_part) / (Bf + 1e-8)

    # ---- focal + label smoothing (host tail) ----
    pred, lab = prep["pred"], prep["lab"]
    se = np.empty(B)
    for c in range(N_CORES):
        for m in range(RT):
            se[c * R + m * 128:c * R + (m + 1) * 128] = \
                accs[c, :, meta["C_SE"] + m]
    lse = np.log(se)
    ptgt = pred[np.arange(B), lab]
    spred = pred.sum(axis=1)
    ce = lse - ptgt
    pt = np.exp(-ce)
    lf = (ALPHA * (1.0 - pt) ** GAMMA * ce).mean()
    ls = (-(OFF * (spred - C * lse)
            + (1.0 - SMOOTHING - OFF) * (ptgt - lse))).mean()

    total = (W_CONTRASTIVE * lc + W_TRIPLET * lt
             + W_FOCAL * lf + W_LABEL_SMOOTH * ls)
    return np.array([lc, lt, lf, ls, total], dtype=np.float32)


def kernel(pred, target, features):
    in_maps, prep = _host_prep(pred, target, features)
    nc, meta = _build(prep["S0"], prep["S1"])
    res = run_bass_kernel_spmd(nc, in_maps, core_ids=list(range(N_CORES)))
    return _combine(res.results, meta, prep)


if __name__ == "__main__":
    import reference

    inputs = reference.setup_inputs()
    expected = np.asarray(reference.reference(**inputs))
    actual = kernel(**{k: np.asarray(v) for k, v in inputs.items()})
    err = np.abs(actual - expected) / np.maximum(np.abs(expected), 1e-12)
    print("expected:", expected)
    print("actual:  ", actual)
    print("rel err: ", err)
